# revision 7
# baseline (speedup 1.0000x reference)
"""Trainium2 Bass kernel for the CLIP text/image concat multi-head classifier.

Full (unsharded) inputs in, full outputs out. Internally the 312 heads are
sharded 39-per-core across 8 NeuronCores (head/expert parallel); outputs are
gathered and concatenated along the class axis on the host.

Per-core device program (SPMD, identical program / different data):
  lin1/lin2:  image @ W_img.T via PE (f32r), per-head text dot via DVE
              multiply-reduce, bias via ACT.
  logits:     unnormalized text.T @ image via PE, row/col norms folded in
              afterwards (per-partition scales + ones-broadcast matmul).
  class1/2:   per-(head,hidden) rows packed 128/partition-tile;
              z = W_img-stationary matmuls over d-chunks (f32r),
              text term per head via PE matvec -> DRAM bounce -> bias column,
              relu+bias on ACT, batch stats via bn_stats/bn_aggr (stats over
              the free/batch axis), then batchnorm+output projection folded
              into one block-diagonal matmul:
                class[n,b] = sum_row A[row,n]*r[row,b] - (sum aμ)[n] + const0[n]
              with A[row,n] = mask[row,n] * w2[row]*gamma[row]*rsqrt(var+eps),
              and the mu column carried as the (B+1)-th column of the r tile.
"""

import os
import sys
from contextlib import ExitStack

for _p in ("/opt/trn_rl_repo", "/root/.axon_site/_ro/trn_rl_repo"):
    if os.path.isdir(_p) and _p not in sys.path:
        sys.path.insert(0, _p)

import numpy as np
import concourse.bass as bass
import concourse.tile as tile
from concourse import bacc, mybir
from concourse.bass_utils import run_bass_kernel_spmd

F32 = mybir.dt.float32
F32R = mybir.dt.float32r
AF = mybir.ActivationFunctionType
MUL = mybir.AluOpType.mult
ADD = mybir.AluOpType.add
ts = bass.ts

B, N, DE, DV, H = 256, 312, 512, 768, 312
EPS = 1e-5
NC = 8
NH = N // NC              # 39 heads per core
ROWS = NH * H             # 12168 (head, hidden) rows per core
RT = (ROWS + 127) // 128  # 96 row tiles
RPAD = RT * 128           # 12288
TAIL = ROWS - 128 * (RT - 1)  # rows in the last tile (8)
C1D = DE // 128           # 4 d-chunks (classifier1 image part / text parts)
C2D = DV // 128           # 6 d-chunks (classifier2 image part)


class Ctx:
    pass


def _load_persistents(nc, tc, ctx, ins):
    c = Ctx()
    const = ctx.enter_context(tc.tile_pool(name="const", bufs=1))
    c.sp = ctx.enter_context(tc.tile_pool(name="sp", bufs=3))

    def ld(name, shape, dt):
        t = const.tile(shape, dt, tag=name)
        nc.sync.dma_start(t[:], ins[name][:])
        return t

    c.imgT = ld("imgT", [128, C1D * B], F32R)
    c.ioutT = ld("ioutT", [128, C2D * B], F32R)
    c.textT = ld("textT", [128, C1D * NH], F32R)
    c.toutT = ld("toutT", [128, C1D * NH], F32R)
    c.w1iT = ld("w1iT", [128, C1D * NH], F32R)
    c.w2iT = ld("w2iT", [128, C2D * NH], F32R)
    c.text_sl = ld("text_sl", [NH, DE], F32)
    c.tout_sl = ld("tout_sl", [NH, DE], F32)
    c.w1t_sl = ld("w1t_sl", [NH, DE], F32)
    c.w2t_sl = ld("w2t_sl", [NH, DE], F32)
    c.lb1 = ld("lb1", [NH, 1], F32)
    c.lb2 = ld("lb2", [NH, 1], F32)
    c.cst1 = ld("cst1", [NH, 1], F32)
    c.cst2 = ld("cst2", [NH, 1], F32)
    c.b1c = ld("b1c", [128, RT], F32)
    c.b2c = ld("b2c", [128, RT], F32)
    c.w2g1 = ld("w2g1", [128, RT], F32)
    c.w2g2 = ld("w2g2", [128, RT], F32)
    c.m0 = ld("m0", [128, RT * NH], F32)
    c.lst = ld("ls", [1, 1], F32)

    c.ones_col = const.tile([128, 1], F32, tag="ones_col")
    nc.vector.memset(c.ones_col[:], 1.0)
    c.ones_row = const.tile([1, NH], F32, tag="ones_row")
    nc.vector.memset(c.ones_row[:], 1.0)
    c.eps_col = const.tile([128, 1], F32, tag="eps_col")
    nc.vector.memset(c.eps_col[:], EPS)

    dramp = ctx.enter_context(tc.tile_pool(name="dram", bufs=1, space="DRAM"))
    c.t1d = dramp.tile([ROWS], F32, tag="t1d")
    c.t2d = dramp.tile([ROWS], F32, tag="t2d")
    return c


def _phase_lin_logits(nc, c, spp, outs):
    sp = c.sp
    # lin1 / lin2
    for (wT, imt, nch, tsl, wsl, lbt, oname) in (
            (c.w1iT, c.imgT, C1D, c.text_sl, c.w1t_sl, c.lb1, "lin1_o"),
            (c.w2iT, c.ioutT, C2D, c.tout_sl, c.w2t_sl, c.lb2, "lin2_o")):
        lp = spp.tile([NH, B], F32, tag="linp", bufs=2)
        for ch in range(nch):
            nc.tensor.matmul(lp[:], wT[:, ts(ch, NH)], imt[:, ts(ch, B)],
                             start=(ch == 0), stop=(ch == nch - 1))
        junk = sp.tile([NH, DE], F32, tag="junk")
        tl = sp.tile([NH, 1], F32, tag="tl")
        nc.vector.tensor_mul(junk[:], tsl[:], wsl[:])
        nc.vector.tensor_reduce(tl[:], junk[:], mybir.AxisListType.X, ADD)
        lbias = sp.tile([NH, 1], F32, tag="lbias")
        nc.vector.tensor_add(lbias[:], tl[:], lbt[:])
        lsb = sp.tile([NH, B], F32, tag="lsb")
        nc.scalar.activation(lsb[:], lp[:], AF.Identity, bias=lbias[:])
        nc.sync.dma_start(outs[oname][:], lsb[:])

    # logits: G = text.T @ image (unnormalized), then fold norms + exp(s)
    gp = spp.tile([NH, B], F32, tag="linp", bufs=2)
    for ch in range(C1D):
        nc.tensor.matmul(gp[:], c.textT[:, ts(ch, NH)], c.imgT[:, ts(ch, B)],
                         start=(ch == 0), stop=(ch == C1D - 1))
    n2 = spp.tile([1, B], F32, tag="n2", bufs=1)
    for ch in range(C1D):
        sq = sp.tile([128, B], F32, tag="sq")
        nc.scalar.square(sq[:], c.imgT[:, ts(ch, B)].bitcast(F32))
        nc.tensor.matmul(n2[:], c.ones_col[:], sq[:],
                         start=(ch == 0), stop=(ch == C1D - 1))
    nrm = sp.tile([1, B], F32, tag="nrm")
    nc.scalar.sqrt(nrm[:], n2[:])
    inv_i = sp.tile([1, B], F32, tag="invi")
    nc.vector.reciprocal(inv_i[:], nrm[:])
    bcp = spp.tile([NH, B], F32, tag="bcp", bufs=1)
    nc.tensor.matmul(bcp[:], c.ones_row[:], inv_i[:], start=True, stop=True)

    junk3 = sp.tile([NH, DE], F32, tag="junk")
    tn2 = sp.tile([NH, 1], F32, tag="tl")
    nc.vector.tensor_mul(junk3[:], c.text_sl[:], c.text_sl[:])
    nc.vector.tensor_reduce(tn2[:], junk3[:], mybir.AxisListType.X, ADD)
    tnr = sp.tile([NH, 1], F32, tag="tnr")
    nc.scalar.sqrt(tnr[:], tn2[:])
    inv_t = sp.tile([NH, 1], F32, tag="invt")
    nc.vector.reciprocal(inv_t[:], tnr[:])

    sbp = spp.tile([NH, 1], F32, tag="sbp", bufs=1)
    nc.tensor.matmul(sbp[:], c.ones_row[:], c.lst[:], start=True, stop=True)
    es = sp.tile([NH, 1], F32, tag="es")
    nc.scalar.activation(es[:], sbp[:], AF.Exp)
    sc = sp.tile([NH, 1], F32, tag="sc")
    nc.vector.tensor_mul(sc[:], es[:], inv_t[:])

    bcs = sp.tile([NH, B], F32, tag="lsb")
    nc.scalar.copy(bcs[:], bcp[:])
    lg = sp.tile([NH, B], F32, tag="lg")
    nc.vector.tensor_mul(lg[:], gp[:], bcs[:])
    nc.vector.tensor_scalar_mul(lg[:], lg[:], sc[:])
    nc.sync.dma_start(outs["lgt_o"][:], lg[:])


def _phase_text_terms(nc, tc, c, ins):
    sp = c.sp
    with tc.tile_pool(name="wtp", bufs=3) as wtp, \
         tc.tile_pool(name="tpp", bufs=2, space="PSUM") as tpp:
        for (wt_in, src, td) in ((ins["wt1"], c.textT, c.t1d),
                                 (ins["wt2"], c.toutT, c.t2d)):
            for n in range(NH):
                wt = wtp.tile([128, C1D * H], F32R, tag="wt")
                nc.sync.dma_start(wt[:], wt_in[n])
                tps = tpp.tile([1, H], F32, tag="tps")
                for ch in range(C1D):
                    nc.tensor.matmul(tps[:], src[:, ch * NH + n: ch * NH + n + 1],
                                     wt[:, ts(ch, H)],
                                     start=(ch == 0), stop=(ch == C1D - 1))
                trow = sp.tile([1, H], F32, tag="trow")
                nc.scalar.copy(trow[:], tps[:])
                nc.sync.dma_start(td[n * H:(n + 1) * H], trow[:])


def _phase_bias_cols(nc, tc, ctx, c):
    biasp = ctx.enter_context(tc.tile_pool(name="biasp", bufs=1))
    full = 128 * (RT - 1)

    def build(td, bct, name):
        tb = biasp.tile([128, RT], F32, tag="tb" + name)
        nc.sync.dma_start(tb[:, :RT - 1],
                          td[:full].rearrange("(t p) -> p t", p=128))
        nc.sync.dma_start(tb[:TAIL, RT - 1:RT],
                          td[full:].rearrange("(t p) -> p t", p=TAIL))
        bias = biasp.tile([128, RT], F32, tag="bias" + name)
        nc.vector.tensor_add(bias[:], tb[:], bct[:])
        return bias

    c.bias1 = build(c.t1d, c.b1c, "1")
    c.bias2 = build(c.t2d, c.b2c, "2")


def _phase_classifiers(nc, tc, c, ins, outs):
    sp = c.sp
    with tc.tile_pool(name="wzp", bufs=4) as wzp, \
         tc.tile_pool(name="rp", bufs=24) as rp, \
         tc.tile_pool(name="zp", bufs=4, space="PSUM") as zp, \
         tc.tile_pool(name="pp", bufs=2, space="PSUM") as pp:
        for (wz_in, nch, biast, w2gt, cstt, out_o) in (
                (ins["wz1"], C1D, c.bias1, c.w2g1, c.cst1, outs["cls1_o"]),
                (ins["wz2"], C2D, c.bias2, c.w2g2, c.cst2, outs["cls2_o"])):
            imt = c.imgT if nch == C1D else c.ioutT
            ppt = pp.tile([NH, B + 1], F32, tag="pp")
            for t in range(RT):
                M = 128 if t < RT - 1 else TAIL
                wz = wzp.tile([128, nch * 128], F32R, tag="wz")
                nc.sync.dma_start(wz[:], wz_in[t])
                zps = zp.tile([128, B], F32, tag="zps")
                for ch in range(nch):
                    nc.tensor.matmul(zps[:M], wz[:, ch * 128: ch * 128 + M],
                                     imt[:, ts(ch, B)],
                                     start=(ch == 0), stop=(ch == nch - 1))
                r = rp.tile([128, B + 1], F32, tag="r")
                nc.scalar.activation(r[:M, :B], zps[:M], AF.Relu,
                                     bias=biast[:M, t:t + 1])
                st6 = sp.tile([128, 6], F32, tag="st6")
                nc.vector.bn_stats(st6[:M], r[:M, :B])
                agg = sp.tile([128, 2], F32, tag="agg")
                nc.vector.bn_aggr(agg[:M], st6[:M])
                sv = sp.tile([128, 1], F32, tag="sv")
                nc.scalar.activation(sv[:M], agg[:M, 1:2], AF.Sqrt,
                                     bias=c.eps_col[:M])
                inv = sp.tile([128, 1], F32, tag="inv")
                nc.vector.reciprocal(inv[:M], sv[:M])
                ac = sp.tile([128, 1], F32, tag="ac")
                nc.vector.tensor_mul(ac[:M], inv[:M], w2gt[:M, t:t + 1])
                At = sp.tile([128, NH], F32, tag="At")
                nc.vector.tensor_scalar_mul(At[:M], c.m0[:M, ts(t, NH)], ac[:M])
                nc.vector.tensor_copy(r[:M, B:B + 1], agg[:M, 0:1])
                nc.tensor.matmul(ppt[:], At[:M], r[:M, :B + 1],
                                 start=(t == 0), stop=(t == RT - 1))
            mcol = sp.tile([NH, 1], F32, tag="mcol")
            nc.vector.tensor_copy(mcol[:], ppt[:, B:B + 1])
            cbias = sp.tile([NH, 1], F32, tag="cbias")
            nc.vector.tensor_sub(cbias[:], cstt[:], mcol[:])
            csb = sp.tile([NH, B], F32, tag="lsb")
            nc.vector.tensor_scalar_add(csb[:], ppt[:, :B], cbias[:])
            nc.sync.dma_start(out_o[:], csb[:])


def _emit_body(nc, tc, ctx, ins, outs):
    PH = int(os.environ.get("KPH", "7"))
    c = _load_persistents(nc, tc, ctx, ins)
    with tc.tile_pool(name="spp", bufs=3, space="PSUM") as spp:
        if PH & 1:
            _phase_lin_logits(nc, c, spp, outs)
        if PH & 2:
            _phase_text_terms(nc, tc, c, ins)
    if PH & 4:
        _phase_bias_cols(nc, tc, ctx, c)
        _phase_classifiers(nc, tc, c, ins, outs)


def _build(loop_k=1):
    nc = bacc.Bacc("TRN2", target_bir_lowering=False, debug=False,
                   num_devices=NC)
    mk = nc.dram_tensor

    def inp(name, shape, dt):
        return mk(name, shape, dt, kind="ExternalInput").ap()

    ins = {
        "imgT": inp("imgT", [128, C1D * B], F32R),
        "ioutT": inp("ioutT", [128, C2D * B], F32R),
        "textT": inp("textT", [128, C1D * NH], F32R),
        "toutT": inp("toutT", [128, C1D * NH], F32R),
        "w1iT": inp("w1iT", [128, C1D * NH], F32R),
        "w2iT": inp("w2iT", [128, C2D * NH], F32R),
        "text_sl": inp("text_sl", [NH, DE], F32),
        "tout_sl": inp("tout_sl", [NH, DE], F32),
        "w1t_sl": inp("w1t_sl", [NH, DE], F32),
        "w2t_sl": inp("w2t_sl", [NH, DE], F32),
        "lb1": inp("lb1", [NH, 1], F32),
        "lb2": inp("lb2", [NH, 1], F32),
        "cst1": inp("cst1", [NH, 1], F32),
        "cst2": inp("cst2", [NH, 1], F32),
        "b1c": inp("b1c", [128, RT], F32),
        "b2c": inp("b2c", [128, RT], F32),
        "w2g1": inp("w2g1", [128, RT], F32),
        "w2g2": inp("w2g2", [128, RT], F32),
        "m0": inp("m0", [128, RT * NH], F32),
        "ls": inp("ls", [1, 1], F32),
        "wz1": inp("wz1", [RT, 128, C1D * 128], F32R),
        "wz2": inp("wz2", [RT, 128, C2D * 128], F32R),
        "wt1": inp("wt1", [NH, 128, C1D * H], F32R),
        "wt2": inp("wt2", [NH, 128, C1D * H], F32R),
    }
    outs = {
        k: mk(k, [NH, B], F32, kind="ExternalOutput").ap()
        for k in ("lin1_o", "lin2_o", "cls1_o", "cls2_o", "lgt_o")
    }

    with tile.TileContext(nc) as tc:
        with ExitStack() as ctx:
            if loop_k > 1:
                with tc.For_i(0, loop_k, 1):
                    _emit_body(nc, tc, ctx, ins, outs)
            else:
                _emit_body(nc, tc, ctx, ins, outs)
    nc.compile()
    return nc


def _pack_T(x, nch):
    # x: [rows, d] -> [128, nch*rows] with element [p, ch*rows + r] = x[r, ch*128+p]
    rows = x.shape[0]
    return np.ascontiguousarray(
        x.T.reshape(nch, 128, rows).transpose(1, 0, 2).reshape(128, nch * rows))


def _pack_rows(w, nch):
    # w: [ROWS, d] -> [RT, 128, nch*128]: el [t, p, ch*128+r] = w[t*128+r, ch*128+p]
    a = np.zeros((RPAD, nch * 128), np.float32)
    a[:ROWS] = w
    return np.ascontiguousarray(
        a.reshape(RT, 128, nch, 128).transpose(0, 3, 2, 1).reshape(RT, 128, nch * 128))


def _pack_cols(v):
    # v: [ROWS] -> [128, RT], column t = v[t*128:(t+1)*128], zero-padded
    vp = np.zeros(RPAD, np.float32)
    vp[:ROWS] = v
    return np.ascontiguousarray(vp.reshape(RT, 128).T)


def host_prep(inputs):
    f32 = np.float32
    g = {k: np.asarray(v, f32) for k, v in inputs.items()}
    image_embed, text_embed = g["image_embed"], g["text_embed"]
    image_out, text_out = g["image_out"], g["text_out"]

    imgT = _pack_T(image_embed, C1D)
    ioutT = _pack_T(image_out, C2D)

    # head(row) mask for the block-diagonal projection (same on every core)
    rowhead = np.arange(RPAD) // H
    m0_flat = (rowhead[:, None] == np.arange(NH)[None]).astype(f32)
    m0 = np.ascontiguousarray(
        m0_flat.reshape(RT, 128, NH).transpose(1, 0, 2).reshape(128, RT * NH))

    in_maps = []
    for c in range(NC):
        S = slice(c * NH, (c + 1) * NH)
        c1w, c2w = g["C1_W1"][S], g["C2_W1"][S]
        wz1 = _pack_rows(c1w[:, :, :DE].reshape(ROWS, DE), C1D)
        wz2 = _pack_rows(c2w[:, :, :DV].reshape(ROWS, DV), C2D)
        c1txt = c1w[:, :, DE:]                      # [NH, H, DE]
        c2txt = c2w[:, :, DV:]                      # [NH, H, DE]
        wt1 = np.ascontiguousarray(
            c1txt.reshape(NH, H, C1D, 128).transpose(0, 3, 2, 1).reshape(NH, 128, C1D * H))
        wt2 = np.ascontiguousarray(
            c2txt.reshape(NH, H, C1D, 128).transpose(0, 3, 2, 1).reshape(NH, 128, C1D * H))

        w2gam1 = (g["C1_W2"][S] * g["C1_gamma"][S]).reshape(ROWS)
        w2gam2 = (g["C2_W2"][S] * g["C2_gamma"][S]).reshape(ROWS)
        cst1 = g["C1_b2"][S] + (g["C1_W2"][S] * g["C1_beta"][S]).sum(1)
        cst2 = g["C2_b2"][S] + (g["C2_W2"][S] * g["C2_beta"][S]).sum(1)

        in_maps.append({
            "imgT": imgT, "ioutT": ioutT,
            "textT": _pack_T(text_embed[S], C1D),
            "toutT": _pack_T(text_out[S], C1D),
            "w1iT": _pack_T(g["W1"][S, :DE], C1D),
            "w2iT": _pack_T(g["W2"][S, :DV], C2D),
            "text_sl": np.ascontiguousarray(text_embed[S]),
            "tout_sl": np.ascontiguousarray(text_out[S]),
            "w1t_sl": np.ascontiguousarray(g["W1"][S, DE:]),
            "w2t_sl": np.ascontiguousarray(g["W2"][S, DV:]),
            "lb1": np.ascontiguousarray(g["b1"][S][:, None]),
            "lb2": np.ascontiguousarray(g["b2"][S][:, None]),
            "cst1": np.ascontiguousarray(cst1[:, None]),
            "cst2": np.ascontiguousarray(cst2[:, None]),
            "b1c": _pack_cols(g["C1_b1"][S].reshape(ROWS)),
            "b2c": _pack_cols(g["C2_b1"][S].reshape(ROWS)),
            "w2g1": _pack_cols(w2gam1),
            "w2g2": _pack_cols(w2gam2),
            "m0": m0,
            "ls": g["logit_scale"].reshape(1, 1),
            "wz1": wz1, "wz2": wz2, "wt1": wt1, "wt2": wt2,
        })
    return in_maps


_cache = {}


def _get_nc(loop_k=1):
    if loop_k not in _cache:
        _cache[loop_k] = _build(loop_k)
    return _cache[loop_k]


def run(inputs, loop_k=1):
    nc = _get_nc(loop_k)
    in_maps = host_prep(inputs)
    res = run_bass_kernel_spmd(nc, in_maps, core_ids=list(range(NC)))
    names = ("lin1_o", "lin2_o", "cls1_o", "cls2_o", "lgt_o")
    full = []
    for nm in names:
        parts = [res.results[c][nm] for c in range(NC)]
        full.append(np.ascontiguousarray(np.concatenate(parts, axis=0).T))
    return tuple(full)


def kernel(**inputs):
    return run(inputs, loop_k=1)


# revision 9
# speedup vs baseline: 1.7181x; 1.7181x over previous
"""Trainium2 Bass kernel for the CLIP text/image concat multi-head classifier.

Full (unsharded) inputs in, full outputs out. The 312 heads are sharded
39-per-core across 8 NeuronCores (head/expert parallel); outputs are gathered
and concatenated along the class axis on the host. No collectives: every
core's outputs are disjoint class slices.

Per-core device program (SPMD, identical program / different data), fp16
weights/activations with fp32 PSUM accumulation:

  lin1/lin2:  image @ W_img.T on PE, per-head text dot via DVE multiply+
              reduce, bias via ACT.
  logits:     unnormalized text.T @ image on PE; row/col norms folded in
              afterwards (per-partition scales + ones-broadcast matmul).
  class1/2:   (head, hidden) rows flattened and tiled 104/partition-tile so
              every tile holds exactly one head (312 = 3*104, 12168 = 117*104,
              no tail, no head straddling). Per tile, one merged weight DMA
              brings both the image-part and text-part weight chunks:
                z[:, :256]  += Wimg_ch.T @ imgT_ch      (nimg chunks)
                z[:, 256]   += Wtxt_ch.T @ text[head]   (4 chunks)
              relu(z + t + b) on ACT, batch stats via bn_stats/bn_aggr over
              the free (batch) axis, then batchnorm + output projection are
              folded into one accumulated block-diagonal matmul:
                class[n,b] = sum_row A[row,n]*r[row,b] - (sum a*mu)[n] + K[n]
              with A[row,n] = a[row] placed in column head(row),
              a = w2*gamma*rsqrt(var+eps), and mu carried as the 257th column
              of the r tile so sum a*mu falls out of the same matmul.
"""

import os
import sys
from contextlib import ExitStack

for _p in ("/opt/trn_rl_repo", "/root/.axon_site/_ro/trn_rl_repo"):
    if os.path.isdir(_p) and _p not in sys.path:
        sys.path.insert(0, _p)

import numpy as np
import concourse.bass as bass
import concourse.tile as tile
from concourse import bacc, mybir
from concourse.bass_utils import run_bass_kernel_spmd

F32 = mybir.dt.float32
F16 = mybir.dt.float16
AF = mybir.ActivationFunctionType
MUL = mybir.AluOpType.mult
ADD = mybir.AluOpType.add
DIV = mybir.AluOpType.divide
ts = bass.ts

B, N, DE, DV, H = 256, 312, 512, 768, 312
EPS = 1e-5
NC = 8
NH = N // NC              # 39 heads per core
ROWS = NH * H             # 12168 (head, hidden) rows per core
TR = 104                  # rows per tile; 312 = 3*TR so tiles never straddle heads
NT = ROWS // TR           # 117 row tiles per classifier
C1D = DE // 128           # 4 contraction chunks (classifier1 image / text parts)
C2D = DV // 128           # 6 contraction chunks (classifier2 image part)


class Ctx:
    pass


def _load_persistents(nc, tc, ctx, ins):
    c = Ctx()
    const = ctx.enter_context(tc.tile_pool(name="const", bufs=1))
    c.sp = ctx.enter_context(tc.tile_pool(name="sp", bufs=3))

    def ld(name, shape, dt):
        t = const.tile(shape, dt, tag=name)
        nc.sync.dma_start(t[:], ins[name][:])
        return t

    c.imgT = ld("imgT", [128, C1D * B], F16)
    c.ioutT = ld("ioutT", [128, C2D * B], F16)
    c.textT = ld("textT", [128, C1D * NH], F16)
    c.toutT = ld("toutT", [128, C1D * NH], F16)
    c.w1iT = ld("w1iT", [128, C1D * NH], F16)
    c.w2iT = ld("w2iT", [128, C2D * NH], F16)
    c.text_sl = ld("text_sl", [NH, DE], F32)
    c.tout_sl = ld("tout_sl", [NH, DE], F32)
    c.w1t_sl = ld("w1t_sl", [NH, DE], F32)
    c.w2t_sl = ld("w2t_sl", [NH, DE], F32)
    c.lb1 = ld("lb1", [NH, 1], F32)
    c.lb2 = ld("lb2", [NH, 1], F32)
    c.cst1 = ld("cst1", [NH, 1], F32)
    c.cst2 = ld("cst2", [NH, 1], F32)
    c.b1c = ld("b1c", [TR, NT], F32)
    c.b2c = ld("b2c", [TR, NT], F32)
    c.w2g1 = ld("w2g1", [TR, NT], F32)
    c.w2g2 = ld("w2g2", [TR, NT], F32)
    c.lst = ld("ls", [1, 1], F32)

    c.ones_col = const.tile([128, 1], F16, tag="ones_col")
    nc.vector.memset(c.ones_col[:], 1.0)
    c.ones_row = const.tile([1, NH], F32, tag="ones_row")
    nc.vector.memset(c.ones_row[:], 1.0)
    c.eps_col = const.tile([128, 1], F32, tag="eps_col")
    nc.vector.memset(c.eps_col[:], EPS)
    return c


def _phase_lin_logits(nc, c, spp, outs):
    sp = c.sp
    # lin1 / lin2
    for (wT, imt, nch, tsl, wsl, lbt, oname) in (
            (c.w1iT, c.imgT, C1D, c.text_sl, c.w1t_sl, c.lb1, "lin1_o"),
            (c.w2iT, c.ioutT, C2D, c.tout_sl, c.w2t_sl, c.lb2, "lin2_o")):
        lp = spp.tile([NH, B], F32, tag="linp", bufs=2)
        for ch in range(nch):
            nc.tensor.matmul(lp[:], wT[:, ts(ch, NH)], imt[:, ts(ch, B)],
                             start=(ch == 0), stop=(ch == nch - 1))
        junk = sp.tile([NH, DE], F32, tag="junk")
        tl = sp.tile([NH, 1], F32, tag="tl")
        nc.vector.tensor_mul(junk[:], tsl[:], wsl[:])
        nc.vector.tensor_reduce(tl[:], junk[:], mybir.AxisListType.X, ADD)
        lbias = sp.tile([NH, 1], F32, tag="lbias")
        nc.vector.tensor_add(lbias[:], tl[:], lbt[:])
        lsb = sp.tile([NH, B], F32, tag="lsb")
        nc.scalar.activation(lsb[:], lp[:], AF.Identity, bias=lbias[:])
        nc.sync.dma_start(outs[oname][:], lsb[:])

    # logits: G = text.T @ image (unnormalized), then fold norms + exp(s)
    gp = spp.tile([NH, B], F32, tag="linp", bufs=2)
    for ch in range(C1D):
        nc.tensor.matmul(gp[:], c.textT[:, ts(ch, NH)], c.imgT[:, ts(ch, B)],
                         start=(ch == 0), stop=(ch == C1D - 1))
    n2 = spp.tile([1, B], F32, tag="n2", bufs=1)
    for ch in range(C1D):
        sq = sp.tile([128, B], F16, tag="sq")
        nc.scalar.square(sq[:], c.imgT[:, ts(ch, B)])
        nc.tensor.matmul(n2[:], c.ones_col[:], sq[:],
                         start=(ch == 0), stop=(ch == C1D - 1))
    nrm = sp.tile([1, B], F32, tag="nrm")
    nc.scalar.sqrt(nrm[:], n2[:])
    inv_i = sp.tile([1, B], F32, tag="invi")
    nc.vector.reciprocal(inv_i[:], nrm[:])
    bcp = spp.tile([NH, B], F32, tag="bcp", bufs=1)
    nc.tensor.matmul(bcp[:], c.ones_row[:], inv_i[:], start=True, stop=True)

    junk3 = sp.tile([NH, DE], F32, tag="junk")
    tn2 = sp.tile([NH, 1], F32, tag="tl")
    nc.vector.tensor_mul(junk3[:], c.text_sl[:], c.text_sl[:])
    nc.vector.tensor_reduce(tn2[:], junk3[:], mybir.AxisListType.X, ADD)
    tnr = sp.tile([NH, 1], F32, tag="tnr")
    nc.scalar.sqrt(tnr[:], tn2[:])
    inv_t = sp.tile([NH, 1], F32, tag="invt")
    nc.vector.reciprocal(inv_t[:], tnr[:])

    sbp = spp.tile([NH, 1], F32, tag="sbp", bufs=1)
    nc.tensor.matmul(sbp[:], c.ones_row[:], c.lst[:], start=True, stop=True)
    es = sp.tile([NH, 1], F32, tag="es")
    nc.scalar.activation(es[:], sbp[:], AF.Exp)
    sc = sp.tile([NH, 1], F32, tag="sc")
    nc.vector.tensor_mul(sc[:], es[:], inv_t[:])

    bcs = sp.tile([NH, B], F32, tag="lsb")
    nc.scalar.copy(bcs[:], bcp[:])
    lg = sp.tile([NH, B], F32, tag="lg")
    nc.vector.tensor_mul(lg[:], gp[:], bcs[:])
    nc.vector.tensor_scalar_mul(lg[:], lg[:], sc[:])
    nc.sync.dma_start(outs["lgt_o"][:], lg[:])


def _phase_classifiers(nc, tc, c, ins, outs):
    sp = c.sp
    with tc.tile_pool(name="wmp", bufs=10) as wmp, \
         tc.tile_pool(name="rp", bufs=16) as rp, \
         tc.tile_pool(name="apool", bufs=1) as apool, \
         tc.tile_pool(name="zp", bufs=4, space="PSUM") as zp, \
         tc.tile_pool(name="pp", bufs=2, space="PSUM") as pp:
        for (wm_in, nimg, ttx, bct, w2gt, cstt, out_o) in (
                (ins["wm1"], C1D, c.textT, c.b1c, c.w2g1, c.cst1, outs["cls1_o"]),
                (ins["wm2"], C2D, c.toutT, c.b2c, c.w2g2, c.cst2, outs["cls2_o"])):
            imt = c.imgT if nimg == C1D else c.ioutT
            ppt = pp.tile([NH, B + 1], F32, tag="pp")
            At = apool.tile([TR, NH], F16, tag="At")
            for t in range(NT):
                n = t // 3
                wm = wmp.tile([128, (nimg + C1D) * TR], F16, tag="wm")
                nc.sync.dma_start(wm[:], wm_in[t])
                zps = zp.tile([TR, B + 1], F32, tag="zps")
                for ch in range(nimg):
                    nc.tensor.matmul(zps[:, :B], wm[:, ts(ch, TR)],
                                     imt[:, ts(ch, B)],
                                     start=(ch == 0), stop=(ch == nimg - 1))
                for ch in range(C1D):
                    nc.tensor.matmul(zps[:, B:B + 1],
                                     wm[:, ts(nimg + ch, TR)],
                                     ttx[:, ch * NH + n: ch * NH + n + 1],
                                     start=(ch == 0), stop=(ch == C1D - 1))
                bias_col = sp.tile([TR, 1], F32, tag="bcol")
                nc.scalar.activation(bias_col[:], zps[:, B:B + 1], AF.Identity,
                                     bias=bct[:, t:t + 1])
                r = rp.tile([TR, B + 1], F16, tag="r")
                nc.scalar.activation(r[:, :B], zps[:, :B], AF.Relu,
                                     bias=bias_col[:])
                st6 = sp.tile([TR, 6], F32, tag="st6")
                nc.vector.bn_stats(st6[:], r[:, :B])
                agg = sp.tile([TR, 2], F32, tag="agg")
                nc.vector.bn_aggr(agg[:], st6[:])
                sv = sp.tile([TR, 1], F32, tag="sv")
                nc.scalar.activation(sv[:], agg[:, 1:2], AF.Sqrt,
                                     bias=c.eps_col[:TR])
                inv = sp.tile([TR, 1], F32, tag="inv")
                nc.vector.reciprocal(inv[:], sv[:])
                ac = sp.tile([TR, 1], F32, tag="ac")
                nc.vector.tensor_mul(ac[:], inv[:], w2gt[:, t:t + 1])
                if t % 3 == 0:
                    nc.vector.memset(At[:], 0.0)
                nc.scalar.copy(At[:, n:n + 1], ac[:])
                nc.scalar.copy(r[:, B:B + 1], agg[:, 0:1])
                nc.tensor.matmul(ppt[:], At[:], r[:],
                                 start=(t == 0), stop=(t == NT - 1))
            mcol = sp.tile([NH, 1], F32, tag="mcol")
            nc.vector.tensor_copy(mcol[:], ppt[:, B:B + 1])
            cbias = sp.tile([NH, 1], F32, tag="cbias")
            nc.vector.tensor_sub(cbias[:], cstt[:], mcol[:])
            csb = sp.tile([NH, B], F32, tag="lsb")
            nc.vector.tensor_scalar_add(csb[:], ppt[:, :B], cbias[:])
            nc.sync.dma_start(out_o[:], csb[:])


def _emit_body(nc, tc, ctx, ins, outs):
    PH = int(os.environ.get("KPH", "7"))
    c = _load_persistents(nc, tc, ctx, ins)
    with tc.tile_pool(name="spp", bufs=3, space="PSUM") as spp:
        if PH & 1:
            _phase_lin_logits(nc, c, spp, outs)
    if PH & 4:
        _phase_classifiers(nc, tc, c, ins, outs)


def _build(loop_k=1):
    nc = bacc.Bacc("TRN2", target_bir_lowering=False, debug=False,
                   num_devices=NC)
    mk = nc.dram_tensor

    def inp(name, shape, dt):
        return mk(name, shape, dt, kind="ExternalInput").ap()

    ins = {
        "imgT": inp("imgT", [128, C1D * B], F16),
        "ioutT": inp("ioutT", [128, C2D * B], F16),
        "textT": inp("textT", [128, C1D * NH], F16),
        "toutT": inp("toutT", [128, C1D * NH], F16),
        "w1iT": inp("w1iT", [128, C1D * NH], F16),
        "w2iT": inp("w2iT", [128, C2D * NH], F16),
        "text_sl": inp("text_sl", [NH, DE], F32),
        "tout_sl": inp("tout_sl", [NH, DE], F32),
        "w1t_sl": inp("w1t_sl", [NH, DE], F32),
        "w2t_sl": inp("w2t_sl", [NH, DE], F32),
        "lb1": inp("lb1", [NH, 1], F32),
        "lb2": inp("lb2", [NH, 1], F32),
        "cst1": inp("cst1", [NH, 1], F32),
        "cst2": inp("cst2", [NH, 1], F32),
        "b1c": inp("b1c", [TR, NT], F32),
        "b2c": inp("b2c", [TR, NT], F32),
        "w2g1": inp("w2g1", [TR, NT], F32),
        "w2g2": inp("w2g2", [TR, NT], F32),
        "ls": inp("ls", [1, 1], F32),
        "wm1": inp("wm1", [NT, 128, (C1D + C1D) * TR], F16),
        "wm2": inp("wm2", [NT, 128, (C2D + C1D) * TR], F16),
    }
    outs = {
        k: mk(k, [NH, B], F32, kind="ExternalOutput").ap()
        for k in ("lin1_o", "lin2_o", "cls1_o", "cls2_o", "lgt_o")
    }

    with tile.TileContext(nc) as tc:
        with ExitStack() as ctx:
            if loop_k > 1:
                with tc.For_i(0, loop_k, 1):
                    _emit_body(nc, tc, ctx, ins, outs)
            else:
                _emit_body(nc, tc, ctx, ins, outs)
    nc.compile()
    return nc


def _pack_T(x, nch, dtype):
    # x: [rows, d] -> [128, nch*rows] with element [p, ch*rows + r] = x[r, ch*128+p]
    rows = x.shape[0]
    return np.ascontiguousarray(
        x.T.reshape(nch, 128, rows).transpose(1, 0, 2).reshape(128, nch * rows)
    ).astype(dtype)


def _pack_rows104(w, nch):
    # w: [ROWS, nch*128] -> [NT, 128, nch*TR]: el [t, p, ch*TR+r] = w[TR*t+r, 128*ch+p]
    return np.ascontiguousarray(
        w.reshape(NT, TR, nch, 128).transpose(0, 3, 2, 1).reshape(NT, 128, nch * TR)
    ).astype(np.float16)


def _pack_cols104(v):
    # v: [ROWS] -> [TR, NT], column t = v[t*TR:(t+1)*TR]
    return np.ascontiguousarray(v.reshape(NT, TR).T.astype(np.float32))


def host_prep(inputs):
    f32 = np.float32
    g = {k: np.asarray(v, f32) for k, v in inputs.items()}
    image_embed, text_embed = g["image_embed"], g["text_embed"]
    image_out, text_out = g["image_out"], g["text_out"]

    imgT = _pack_T(image_embed, C1D, np.float16)
    ioutT = _pack_T(image_out, C2D, np.float16)

    in_maps = []
    for c in range(NC):
        S = slice(c * NH, (c + 1) * NH)
        # merged per-row weights: [img chunks | text chunks] is exactly the
        # original concat layout of C*_W1 rows
        wm1 = _pack_rows104(g["C1_W1"][S].reshape(ROWS, DE + DE), C1D + C1D)
        wm2 = _pack_rows104(g["C2_W1"][S].reshape(ROWS, DV + DE), C2D + C1D)

        w2gam1 = (g["C1_W2"][S] * g["C1_gamma"][S]).reshape(ROWS)
        w2gam2 = (g["C2_W2"][S] * g["C2_gamma"][S]).reshape(ROWS)
        cst1 = g["C1_b2"][S] + (g["C1_W2"][S] * g["C1_beta"][S]).sum(1)
        cst2 = g["C2_b2"][S] + (g["C2_W2"][S] * g["C2_beta"][S]).sum(1)

        in_maps.append({
            "imgT": imgT, "ioutT": ioutT,
            "textT": _pack_T(text_embed[S], C1D, np.float16),
            "toutT": _pack_T(text_out[S], C1D, np.float16),
            "w1iT": _pack_T(g["W1"][S, :DE], C1D, np.float16),
            "w2iT": _pack_T(g["W2"][S, :DV], C2D, np.float16),
            "text_sl": np.ascontiguousarray(text_embed[S]),
            "tout_sl": np.ascontiguousarray(text_out[S]),
            "w1t_sl": np.ascontiguousarray(g["W1"][S, DE:]),
            "w2t_sl": np.ascontiguousarray(g["W2"][S, DV:]),
            "lb1": np.ascontiguousarray(g["b1"][S][:, None]),
            "lb2": np.ascontiguousarray(g["b2"][S][:, None]),
            "cst1": np.ascontiguousarray(cst1[:, None]),
            "cst2": np.ascontiguousarray(cst2[:, None]),
            "b1c": _pack_cols104(g["C1_b1"][S].reshape(ROWS)),
            "b2c": _pack_cols104(g["C2_b1"][S].reshape(ROWS)),
            "w2g1": _pack_cols104(w2gam1),
            "w2g2": _pack_cols104(w2gam2),
            "ls": g["logit_scale"].reshape(1, 1),
            "wm1": wm1, "wm2": wm2,
        })
    return in_maps


_cache = {}


def _get_nc(loop_k=1):
    if loop_k not in _cache:
        _cache[loop_k] = _build(loop_k)
    return _cache[loop_k]


def run(inputs, loop_k=1):
    nc = _get_nc(loop_k)
    in_maps = host_prep(inputs)
    res = run_bass_kernel_spmd(nc, in_maps, core_ids=list(range(NC)))
    names = ("lin1_o", "lin2_o", "cls1_o", "cls2_o", "lgt_o")
    full = []
    for nm in names:
        parts = [res.results[c][nm] for c in range(NC)]
        full.append(np.ascontiguousarray(np.concatenate(parts, axis=0).T))
    return tuple(full)


def kernel(**inputs):
    return run(inputs, loop_k=1)
